# revision 32
# baseline (speedup 1.0000x reference)
import sys

if "/opt/trn_rl_repo" not in sys.path:
    sys.path.insert(0, "/opt/trn_rl_repo")

import numpy as np
import ml_dtypes
import concourse.bacc as bacc
import concourse.bass as bass
import concourse.mybir as mybir
import concourse.tile as tile
from concourse.bass_utils import run_bass_kernel_spmd
from concourse.masks import make_identity

# Problem dims (hardcoded per spec)
DIM = 2048
DMEDIA = 1024
HEADS = 16
DH = 64
INNER = 1024
FF = 8192
LAT = 64
B = 4
NTOK = 2048
T = 1024          # tokens per core (one batch element, half its tokens)
P = 128
EPS = 1e-5
NCORES = 8

DC = DIM // P       # 16
DCP = DC // 2       # 8 dim-chunk pairs (DoubleRow)
MC = DMEDIA // P    # 8
IC = INNER // P     # 8
ICP = IC // 2       # 4 inner-chunk pairs
FC = FF // P        # 64
FCP = FC // 2       # 32 ffn-chunk pairs
TS = T // P         # 8 token sub-tiles
DS = 4              # 512-wide output-dim slabs
SCALE = DH ** -0.5

SW = 1024.0         # fp8 scale on Wq/Wo/W1
S2 = 2048.0         # fp8 scale on W2

F32 = mybir.dt.float32
BF16 = mybir.dt.bfloat16
FP8 = mybir.dt.float8e4
AF = mybir.ActivationFunctionType
ALU = mybir.AluOpType
DR = mybir.MatmulPerfMode.DoubleRow

NPBF = ml_dtypes.bfloat16
NPF8 = ml_dtypes.float8_e4m3


def build_program():
    nc = bacc.Bacc("TRN2", target_bir_lowering=False, debug=False)

    xb_d = nc.dram_tensor("xb", [T, DIM], BF16, kind="ExternalInput")
    mediaT_d = nc.dram_tensor("mediaT", [MC, P, LAT], BF16, kind="ExternalInput")
    wkv_d = nc.dram_tensor("wkv", [P, MC * 2 * INNER], BF16, kind="ExternalInput")
    wq_d = nc.dram_tensor("wq8", [DCP, P, 2 * INNER], FP8, kind="ExternalInput")
    wo_d = nc.dram_tensor("wo8", [ICP * DS, P, 2 * 512], FP8, kind="ExternalInput")
    w1_d = nc.dram_tensor("w1", [FC, P, DCP * 2 * P], FP8, kind="ExternalInput")
    w2_d = nc.dram_tensor("w2", [DS, FCP // 4, P, 4 * 2 * 512], FP8,
                          kind="ExternalInput")
    masklog_d = nc.dram_tensor("masklog", [LAT, 1], F32, kind="ExternalInput")
    g1s_d = nc.dram_tensor("g1s", [P, DC], F32, kind="ExternalInput")
    b1s_d = nc.dram_tensor("b1s", [P, DC], F32, kind="ExternalInput")
    g2s_d = nc.dram_tensor("g2s", [P, DC], F32, kind="ExternalInput")
    b2s_d = nc.dram_tensor("b2s", [P, DC], F32, kind="ExternalInput")
    c1_d = nc.dram_tensor("c1", [1, 1], F32, kind="ExternalInput")
    c2_d = nc.dram_tensor("c2", [1, 1], F32, kind="ExternalInput")
    sumsel_d = nc.dram_tensor("sumsel", [P, 2], BF16, kind="ExternalInput")
    onehot_d = nc.dram_tensor("onehot", [2, P], BF16, kind="ExternalInput")
    out_d = nc.dram_tensor("out", [T, DIM], F32, kind="ExternalOutput")
    x1_dram = nc.dram_tensor("x1s", [T, DIM], BF16)  # internal spill

    from contextlib import ExitStack

    with tile.TileContext(nc) as tc, ExitStack() as es_pp:
        # pool stack (LIFO): pp > w1st > qn2T8 > xb > oT8 > qT > qnT8
        #                    > wq8 > wkv
        pp = es_pp.enter_context(tc.tile_pool(name="persist", bufs=1))
        ident = pp.tile([P, P], F32)
        make_identity(nc, ident)
        ident_bf = pp.tile([P, P], BF16)
        make_identity(nc, ident_bf)
        eps_sb = pp.tile([P, 1], F32)
        nc.vector.memset(eps_sb, EPS)
        c1_sb = pp.tile([P, 1], F32)
        c2_sb = pp.tile([P, 1], F32)
        nc.scalar.dma_start(c1_sb[:], bass.AP(
            tensor=c1_d.ap().tensor, offset=0, ap=[[0, P], [1, 1]]))
        nc.scalar.dma_start(c2_sb[:], bass.AP(
            tensor=c2_d.ap().tensor, offset=0, ap=[[0, P], [1, 1]]))
        mask_sb = pp.tile([P, 1], F32)
        nc.scalar.dma_start(mask_sb[0:LAT, :], masklog_d[:])
        nc.scalar.dma_start(mask_sb[LAT:P, :], masklog_d[:])
        g1s_sb = pp.tile([P, DC], F32)
        b1s_sb = pp.tile([P, DC], F32)
        g2s_sb = pp.tile([P, DC], F32)
        b2s_sb = pp.tile([P, DC], F32)
        nc.scalar.dma_start(g1s_sb[:], g1s_d[:])
        nc.scalar.dma_start(b1s_sb[:], b1s_d[:])
        nc.scalar.dma_start(g2s_sb[:], g2s_d[:])
        nc.scalar.dma_start(b2s_sb[:], b2s_d[:])
        sumsel2 = pp.tile([P, 2], BF16)
        nc.scalar.dma_start(sumsel2[:], sumsel_d[:])
        onehot2 = pp.tile([2, P], BF16)
        nc.scalar.dma_start(onehot2[:], onehot_d[:])
        kT_sb = pp.tile([P, IC, LAT], BF16)     # kT: row hh*64+dh of chunk ic
        v2_sb = pp.tile([P, IC, DH], BF16)      # v: row hh*64+lat, head 2ic+hh

        es_w1st = ExitStack()
        w1st = es_w1st.enter_context(tc.tile_pool(name="w1_st", bufs=1))
        w1ring = [w1st.tile([P, DCP, 2, P], FP8, name=f"w1r{i}")
                  for i in range(4)]

        es_q28 = ExitStack()
        q28p = es_q28.enter_context(tc.tile_pool(name="qn2T8_pool", bufs=1))
        qn2T8 = q28p.tile([P, DCP, 2, T], FP8)

        es_xb = ExitStack()
        xbp = es_xb.enter_context(tc.tile_pool(name="xb_pool", bufs=TS))
        xb = [xbp.tile([P, DIM], BF16, tag="xb", name=f"xb{i}")
              for i in range(TS)]

        es_oT = ExitStack()
        oTp = es_oT.enter_context(tc.tile_pool(name="oT_pool", bufs=1))
        oT8 = oTp.tile([P, ICP, 2, T], FP8)

        es_qT = ExitStack()
        qTp = es_qT.enter_context(tc.tile_pool(name="qT_pool", bufs=IC))
        qT = [qTp.tile([P, T], BF16, tag="qT", name=f"qT{i}")
              for i in range(IC)]

        es_A = ExitStack()
        k32p = es_A.enter_context(tc.tile_pool(name="k32_pool", bufs=1))
        ps_a = es_A.enter_context(
            tc.tile_pool(name="ps_a", bufs=2, space="PSUM"))
        ps_kt = es_A.enter_context(
            tc.tile_pool(name="ps_kt", bufs=1, space="PSUM"))

        es_qnT = ExitStack()
        qnTp = es_qnT.enter_context(tc.tile_pool(name="qnT8_pool", bufs=1))
        qnT8 = qnTp.tile([P, DCP, 2, T], FP8)

        es_wq = ExitStack()
        wqp = es_wq.enter_context(tc.tile_pool(name="wq_pool", bufs=1))
        wq8_sb = wqp.tile([P, DCP, 2, INNER], FP8)

        es_wkv = ExitStack()
        wkvp = es_wkv.enter_context(tc.tile_pool(name="wkv_pool", bufs=1))
        wkv_sb = wkvp.tile([P, MC, 2 * INNER], BF16)
        mediaT = wkvp.tile([P, MC, LAT], BF16)

        # prologue DMAs spread across the three rings for earliest PE start
        for mc in range(MC):
            nc.scalar.dma_start(mediaT[:, mc, :], mediaT_d[mc])
        for mc in (0, 1, 2, 3):
            nc.sync.dma_start(wkv_sb[:, mc, :],
                              wkv_d[:, mc * 2048:(mc + 1) * 2048])
        for mc in (4, 5, 6, 7):
            nc.scalar.dma_start(wkv_sb[:, mc, :],
                                wkv_d[:, mc * 2048:(mc + 1) * 2048])
        for ts_ in range(TS):
            nc.gpsimd.dma_start(xb[ts_][:], xb_d[ts_ * P:(ts_ + 1) * P, :])
        for dcp in range(DCP):
            nc.gpsimd.dma_start(wq8_sb[:, dcp, :, :], wq_d[dcp])

        # ---- Phase A matmuls: K/V projections (mc order ~= DMA arrival)
        MCORD = [0, 4, 1, 5, 2, 6, 3, 7]
        k32_sb = k32p.tile([LAT, INNER], F32)
        pk_jh = []
        for jh in range(2):
            js = slice(jh * 512, (jh + 1) * 512)
            pk = ps_a.tile([P, 512], F32, tag="psa", name=f"pk{jh}")
            for n, mc in enumerate(MCORD):
                nc.tensor.matmul(
                    pk[0:LAT, :], mediaT[:, mc, :],
                    wkv_sb[:, mc, 0:INNER][:, js],
                    start=(n == 0), stop=(n == MC - 1))
            nc.vector.tensor_copy(k32_sb[:, js], pk[0:LAT, :])
            pk_jh.append(pk)
        pv_jh = []
        for jh in range(2):
            js = slice(INNER + jh * 512, INNER + (jh + 1) * 512)
            pv = ps_a.tile([P, 512], F32, tag="psa", name=f"pv{jh}")
            for hh in range(2):
                po = hh * LAT
                for n, mc in enumerate(MCORD):
                    nc.tensor.matmul(
                        pv[po:po + LAT, :], mediaT[:, mc, :],
                        wkv_sb[:, mc, js],
                        start=(n == 0), stop=(n == MC - 1))
            pv_jh.append(pv)
        es_wkv.close()

        # ------- Phases B+C (+A drain): LN1 -> qnT8; Q proj (fp8 DR) ----
        with tc.tile_pool(name="qt_pool", bufs=5) as qtp, \
             tc.tile_pool(name="stats", bufs=8) as stp, \
             tc.tile_pool(name="ps_tr", bufs=2, space="PSUM") as ps_tr, \
             tc.tile_pool(name="ps_q", bufs=2, space="PSUM") as ps_q:
            for grp in range(2):
                qts = []
                for i2 in range(4):
                    ts_ = grp * 4 + i2
                    st = stp.tile([P, 4, 6], F32, tag="st")
                    for j in range(4):
                        nc.vector.bn_stats(
                            st[:, j, :], xb[ts_][:, j * 512:(j + 1) * 512])
                    mv = stp.tile([P, 2], F32, tag="mv")
                    nc.vector.bn_aggr(mv[:], st[:])
                    rstd = stp.tile([P, 1], F32, tag="rstd")
                    nc.scalar.activation(
                        rstd[:], mv[:, 1:2], AF.Sqrt, bias=eps_sb[:])
                    nc.vector.reciprocal_approx_fast(rstd[:], rstd[:])
                    qt = qtp.tile([P, DIM], BF16, tag="qt")
                    nc.vector.tensor_scalar(
                        qt[:], xb[ts_][:],
                        scalar1=mv[:, 0:1], scalar2=rstd[:],
                        op0=ALU.subtract, op1=ALU.mult)
                    qts.append(qt)
                for c in range(DC):
                    pt = ps_tr.tile([P, 512], BF16, tag="tr")
                    for i2 in range(4):
                        nc.tensor.transpose(
                            pt[:, i2 * P:(i2 + 1) * P],
                            qts[i2][:, c * P:(c + 1) * P], ident_bf[:])
                    nc.scalar.activation(
                        qnT8[:, c // 2, c % 2, grp * 512:(grp + 1) * 512],
                        pt[:], AF.Identity,
                        bias=b1s_sb[:, c:c + 1], scale=g1s_sb[:, c:c + 1])
                if grp == 0:
                    # A drain: kT via PE transpose; v2 gather copies
                    for ic in range(IC):
                        pt = ps_kt.tile([P, LAT], F32, tag="kt")
                        nc.tensor.transpose(
                            pt[:, :], k32_sb[:, ic * P:(ic + 1) * P],
                            ident[:LAT, :LAT])
                        nc.vector.tensor_copy(kT_sb[:, ic, :], pt[:])
                    for jh in range(2):
                        for hh in range(2):
                            po = hh * LAT
                            nc.vector.tensor_copy(
                                v2_sb[po:po + LAT, 4 * jh:4 * jh + 4, :],
                                pv_jh[jh][po:po + LAT, :].rearrange(
                                    "l (ic two q) -> l ic two q",
                                    two=2, q=DH)[:, :, hh, :])
                ths = slice(grp * 512, (grp + 1) * 512)
                for ic in range(IC):
                    pq = ps_q.tile([P, 512], F32, tag="q")
                    for dcp in range(DCP):
                        nc.tensor.matmul(
                            pq[:], wq8_sb[:, dcp, :, ic * P:(ic + 1) * P],
                            qnT8[:, dcp, :, ths],
                            start=(dcp == 0), stop=(dcp == DCP - 1),
                            perf_mode=DR)
                    nc.scalar.activation(qT[ic][:, ths], pq[:], AF.Copy,
                                         scale=1.0 / SW)
        es_wq.close()
        es_qnT.close()
        es_A.close()

        # ---------------- Phase D: attention ----------------------------
        with tc.tile_pool(name="attnT_pool", bufs=IC) as atp, \
             tc.tile_pool(name="rp_pool", bufs=2) as rpp:
            at = [atp.tile([P, T], BF16, tag="attnT", name=f"attnT{i}")
                  for i in range(IC)]
            with tc.tile_pool(name="ps_at", bufs=3, space="PSUM") as ps_at:
                for ic in range(IC):
                    ps = ps_at.tile([P, T], F32, tag="at")
                    for hh in range(2):
                        po = hh * LAT
                        for th in range(2):
                            ths = slice(th * 512, (th + 1) * 512)
                            nc.tensor.matmul(
                                ps[po:po + LAT, ths],
                                kT_sb[po:po + LAT, ic, :],
                                qT[ic][po:po + LAT, ths],
                                start=True, stop=True)
                    nc.scalar.activation(at[ic][:], ps[:], AF.Exp,
                                         bias=mask_sb[:], scale=SCALE)
            with tc.tile_pool(name="ps_s2", bufs=2, space="PSUM") as ps_s2, \
                 tc.tile_pool(name="ps_b", bufs=2, space="PSUM") as ps_b:
                for ic in range(IC):
                    ps2 = ps_s2.tile([2, T], F32, tag="s2")
                    for th in range(2):
                        ths = slice(th * 512, (th + 1) * 512)
                        nc.tensor.matmul(ps2[:, ths], sumsel2[:],
                                         at[ic][:, ths],
                                         start=True, stop=True)
                    rp32 = rpp.tile([2, T], F32, tag="rp32")
                    nc.vector.reciprocal_approx_fast(rp32[:], ps2[:])
                    rp = rpp.tile([2, T], BF16, tag="rp")
                    with nc.allow_low_precision(
                            reason="softmax 1/sumexp in bf16; tol 2e-2"):
                        nc.vector.tensor_copy(rp[:], rp32[:])
                    pb = ps_b.tile([P, T], F32, tag="b")
                    for th in range(2):
                        ths = slice(th * 512, (th + 1) * 512)
                        nc.tensor.matmul(pb[:, ths], onehot2[:], rp[:, ths],
                                         start=True, stop=True)
                    nc.vector.tensor_mul(at[ic][:], at[ic][:], pb[:])
            with tc.tile_pool(name="ps_av", bufs=3, space="PSUM") as ps_av:
                for ic in range(IC):
                    pav = ps_av.tile([P, T], F32, tag="av")
                    for hh in range(2):
                        po = hh * LAT
                        for th in range(2):
                            ths = slice(th * 512, (th + 1) * 512)
                            nc.tensor.matmul(
                                pav[po:po + LAT, ths],
                                v2_sb[po:po + LAT, ic, :],
                                at[ic][po:po + LAT, ths],
                                start=True, stop=True)
                    nc.scalar.copy(oT8[:, ic // 2, ic % 2, :], pav[:])
        es_qT.close()

        # ---------------- Phases E+F: O-proj (fp8 DR), LN2, qn2T8 -------
        with tc.tile_pool(name="wo_st", bufs=1) as wost, \
             tc.tile_pool(name="x1_pool", bufs=TS) as x1p, \
             tc.tile_pool(name="t1_pool", bufs=3) as t1p, \
             tc.tile_pool(name="qt2_pool", bufs=5) as qt2p, \
             tc.tile_pool(name="stats2", bufs=8) as st2p:
            x1t = [x1p.tile([P, DIM], BF16, tag="x1", name=f"x1_{i}")
                   for i in range(TS)]
            st2 = [st2p.tile([P, 4, 6], F32, name=f"st2_{i}")
                   for i in range(TS)]
            wotiles = [wost.tile([P, 2, 512], FP8, name=f"wo{i}")
                       for i in range(ICP * DS)]
            for i in range(ICP * DS):
                nc.gpsimd.dma_start(wotiles[i][:], wo_d[i])
            with tc.tile_pool(name="ps_e", bufs=8, space="PSUM") as ps_e:
                for d4 in range(DS):
                    sl = slice(d4 * 512, (d4 + 1) * 512)
                    pos_e = [ps_e.tile([P, 512], F32, tag="e",
                                       name=f"pe{d4}_{i}") for i in range(TS)]
                    for icp in range(ICP):
                        wot = wotiles[icp * DS + d4]
                        for ts_ in range(TS):
                            nc.tensor.matmul(
                                pos_e[ts_],
                                oT8[:, icp, :, ts_ * P:(ts_ + 1) * P],
                                wot[:],
                                start=(icp == 0), stop=(icp == ICP - 1),
                                perf_mode=DR)
                    for ts_ in range(TS):
                        t1 = t1p.tile([P, 512], BF16, tag="t1")
                        nc.scalar.activation(t1[:], pos_e[ts_], AF.Copy,
                                             scale=c1_sb[:])
                        nc.vector.tensor_add(
                            x1t[ts_][:, sl], t1[:], xb[ts_][:, sl])
                        nc.vector.bn_stats(
                            st2[ts_][:, d4, :], x1t[ts_][:, sl])
            # prefetch first W1 tiles before the gpsimd copy burst below
            for i in range(4):
                nc.gpsimd.dma_start(w1ring[i][:], w1_d[i])
            with tc.tile_pool(name="ps_tr2", bufs=2, space="PSUM") as ps_tr2:
                for grp in range(2):
                    q2ts = []
                    for i2 in range(4):
                        ts_ = grp * 4 + i2
                        mv = st2p.tile([P, 2], F32, tag="mv2")
                        nc.vector.bn_aggr(mv[:], st2[ts_][:])
                        rstd = st2p.tile([P, 1], F32, tag="rstd2")
                        nc.scalar.activation(
                            rstd[:], mv[:, 1:2], AF.Sqrt, bias=eps_sb[:])
                        nc.vector.reciprocal_approx_fast(rstd[:], rstd[:])
                        q2t = qt2p.tile([P, DIM], BF16, tag="qt2")
                        nc.vector.tensor_scalar(
                            q2t[:], x1t[ts_][:],
                            scalar1=mv[:, 0:1], scalar2=rstd[:],
                            op0=ALU.subtract, op1=ALU.mult)
                        q2ts.append(q2t)
                        nc.sync.dma_start(
                            x1_dram[ts_ * P:(ts_ + 1) * P, :], x1t[ts_][:])
                    for c in range(DC):
                        pt = ps_tr2.tile([P, 512], BF16, tag="tr2")
                        for i2 in range(4):
                            nc.tensor.transpose(
                                pt[:, i2 * P:(i2 + 1) * P],
                                q2ts[i2][:, c * P:(c + 1) * P], ident_bf[:])
                        nc.scalar.activation(
                            qn2T8[:, c // 2, c % 2,
                                  grp * 512:(grp + 1) * 512], pt[:],
                            AF.Identity,
                            bias=b2s_sb[:, c:c + 1],
                            scale=g2s_sb[:, c:c + 1])
        es_oT.close()
        es_xb.close()

        # ---------------- Phase G: FFN1 (fp8 DR) -> h1T8 ----------------
        es_h1 = ExitStack()
        h1p = es_h1.enter_context(tc.tile_pool(name="h1_pool", bufs=1))
        h1T8 = h1p.tile([P, FCP, 2, T], FP8)
        es_w2st = ExitStack()
        w2st = es_w2st.enter_context(tc.tile_pool(name="w2_st", bufs=1))
        w2ring = [w2st.tile([P, 4, 2, 512], FP8, name=f"w2r{i}")
                  for i in range(4)]
        SG = 1.0 / SW
        with tc.tile_pool(name="ps_g", bufs=4, space="PSUM") as ps_g:
            for fc in range(FC):
                w1t = w1ring[fc % 4]
                for th in range(2):
                    pg = ps_g.tile([P, 512], F32, tag="g")
                    for dcp in range(DCP):
                        nc.tensor.matmul(
                            pg[:], w1t[:, dcp, :, :],
                            qn2T8[:, dcp, :, th * 512:(th + 1) * 512],
                            start=(dcp == 0), stop=(dcp == DCP - 1),
                            perf_mode=DR)
                    nc.scalar.activation(
                        h1T8[:, fc // 2, fc % 2, th * 512:(th + 1) * 512],
                        pg[:], AF.Gelu, scale=SG)
                if fc + 4 < FC:
                    nc.gpsimd.dma_start(w1t[:], w1_d[fc + 4])
                elif fc == FC - 4:
                    nc.gpsimd.dma_start(w2ring[0][:], w2_d[0, 0])
                elif fc == FC - 3:
                    nc.gpsimd.dma_start(w2ring[1][:], w2_d[0, 1])

        # ---------------- Phase H: FFN2 (fp8 DR) + residual -------------
        with tc.tile_pool(name="x1r_pool", bufs=1) as x1rp, \
             tc.tile_pool(name="outst", bufs=4) as outp, \
             tc.tile_pool(name="ps_f2", bufs=8, space="PSUM") as ps_f2:
            x1r = [x1rp.tile([P, 512], BF16, name=f"x1r{i}")
                   for i in range(DS * TS)]
            for i in range(DS * TS):
                ds, ts_ = i // TS, i % TS
                nc.sync.dma_start(
                    x1r[i][:], x1_dram[ts_ * P:(ts_ + 1) * P,
                                       ds * 512:(ds + 1) * 512])
            NG4 = FCP // 4
            for ds in range(DS):
                pos = [ps_f2.tile([P, 512], F32, tag="f2",
                                  name=f"pos{ds}_{i}") for i in range(TS)]
                for g4 in range(NG4):
                    gi = ds * NG4 + g4
                    w2t = w2ring[gi % 4]
                    for i4 in range(4):
                        fcp = g4 * 4 + i4
                        for ts_ in range(TS):
                            nc.tensor.matmul(
                                pos[ts_],
                                h1T8[:, fcp, :, ts_ * P:(ts_ + 1) * P],
                                w2t[:, i4, :, :],
                                start=(fcp == 0), stop=(fcp == FCP - 1),
                                perf_mode=DR)
                    ni = gi + 2
                    if ni < DS * NG4:
                        nc.gpsimd.dma_start(
                            w2ring[ni % 4][:], w2_d[ni // NG4, ni % NG4])
                for ts_ in range(TS):
                    ot = outp.tile([P, 512], F32, tag="out")
                    nc.scalar.activation(ot[:], pos[ts_], AF.Copy,
                                         scale=c2_sb[:])
                    nc.vector.tensor_add(ot[:], ot[:], x1r[ds * TS + ts_][:])
                    dma_eng = nc.scalar if ts_ % 2 == 0 else nc.sync
                    dma_eng.dma_start(
                        out_d[ts_ * P:(ts_ + 1) * P, ds * 512:(ds + 1) * 512],
                        ot[:])
        es_w2st.close()
        es_h1.close()
        es_q28.close()
        es_w1st.close()

    nc.compile()
    return nc


_CACHED_PROG = None
_CACHED_WEIGHTS = None
_CACHED_WID = None


def _get_program():
    global _CACHED_PROG
    if _CACHED_PROG is None:
        _CACHED_PROG = build_program()
    return _CACHED_PROG


def _q8(a, s):
    return np.clip(a * s, -240, 240).astype(NPF8)


def _prep_weights(inputs):
    """Host-side weight prep: cast/tile/transpose into kernel layouts."""
    wq = np.asarray(inputs["Wq"], dtype=np.float32)
    wkv = np.asarray(inputs["Wkv"], dtype=np.float32)
    wo = np.asarray(inputs["Wo"], dtype=np.float32)
    w1 = np.asarray(inputs["W1"], dtype=np.float32)
    w2 = np.asarray(inputs["W2"], dtype=np.float32)
    g1 = np.asarray(inputs["ln_q_g"], dtype=np.float32)
    b1 = np.asarray(inputs["ln_q_b"], dtype=np.float32)
    g2 = np.asarray(inputs["ln_ff_g"], dtype=np.float32)
    b2 = np.asarray(inputs["ln_ff_b"], dtype=np.float32)

    wkv_h = np.ascontiguousarray(
        wkv.reshape(MC, P, 2 * INNER).transpose(1, 0, 2).reshape(P, MC * 2 * INNER)
    ).astype(NPBF)
    # wq8[dcp, p, kt*INNER + i] = Wq[(2*dcp+kt)*128+p, i] * SW
    wq_h = np.ascontiguousarray(
        _q8(wq, SW).reshape(DCP, 2, P, INNER).transpose(0, 2, 1, 3)
        .reshape(DCP, P, 2 * INNER))
    # wo8[icp*DS+d4, p, kt*512 + j] = Wo[(2*icp+kt)*128+p, d4*512+j] * SW
    wo_h = np.ascontiguousarray(
        _q8(wo, SW).reshape(ICP, 2, P, DS, 512).transpose(0, 3, 2, 1, 4)
        .reshape(ICP * DS, P, 2 * 512))
    # w1[fc, p, (dcp,kt,f)] = W1[(2*dcp+kt)*128+p, fc*128+f] * SW
    w1_h = np.ascontiguousarray(
        _q8(w1, SW).reshape(DCP, 2, P, FC, P).transpose(3, 2, 0, 1, 4)
        .reshape(FC, P, DCP * 2 * P))
    # w2[ds, g4, p, (i4,kt,j)] = W2[(2*(4*g4+i4)+kt)*128+p, ds*512+j] * S2
    w2_h = np.ascontiguousarray(
        _q8(w2, S2).reshape(FCP // 4, 4, 2, P, DS, 512).transpose(4, 0, 3, 1, 2, 5)
        .reshape(DS, FCP // 4, P, 4 * 2 * 512))

    # LN affine tiles: [p, dc] = val[dc*128+p] (no scale folds)
    g1s_h = np.ascontiguousarray(g1.reshape(DC, P).T)
    b1s_h = np.ascontiguousarray(b1.reshape(DC, P).T)
    g2s_h = np.ascontiguousarray(g2.reshape(DC, P).T)
    b2s_h = np.ascontiguousarray(b2.reshape(DC, P).T)

    c1 = (np.tanh(np.asarray(inputs["attn_gate"], dtype=np.float32)) / SW
          ).reshape(1, 1)
    c2 = (np.tanh(np.asarray(inputs["ff_gate"], dtype=np.float32)) / S2
          ).reshape(1, 1)

    sumsel = np.zeros((P, 2), dtype=NPBF)
    sumsel[:LAT, 0] = 1.0
    sumsel[LAT:, 1] = 1.0
    onehot = np.ascontiguousarray(sumsel.T)

    return {
        "wq8": wq_h, "wkv": wkv_h, "wo8": wo_h, "w1": w1_h, "w2": w2_h,
        "g1s": g1s_h, "b1s": b1s_h, "g2s": g2s_h, "b2s": b2s_h,
        "c1": c1, "c2": c2, "sumsel": sumsel, "onehot": onehot,
    }


def kernel(**inputs):
    global _CACHED_WEIGHTS, _CACHED_WID
    x = np.asarray(inputs["x"], dtype=np.float32)
    media = np.asarray(inputs["media"], dtype=np.float32)
    mask = np.asarray(inputs["media_mask"])

    wid = tuple(id(inputs[k]) for k in ("Wq", "Wkv", "Wo", "W1", "W2"))
    if _CACHED_WEIGHTS is None or _CACHED_WID != wid:
        _CACHED_WEIGHTS = _prep_weights(inputs)
        _CACHED_WID = wid
    wts = _CACHED_WEIGHTS

    nc = _get_program()
    xb_all = x.astype(NPBF)
    in_maps = []
    for core in range(NCORES):
        b = core // 2
        half = core % 2
        masklog = np.where(mask[b], 0.0, -50.0).astype(np.float32).reshape(LAT, 1)
        mediaT = np.ascontiguousarray(media[b].T.reshape(MC, P, LAT)).astype(NPBF)
        in_maps.append({
            "xb": np.ascontiguousarray(xb_all[b, half * T:(half + 1) * T, :]),
            "mediaT": mediaT,
            "masklog": masklog,
            **wts,
        })
    res = run_bass_kernel_spmd(nc, in_maps, core_ids=list(range(NCORES)))
    out = np.empty((B, NTOK, DIM), dtype=np.float32)
    for core in range(NCORES):
        b = core // 2
        half = core % 2
        out[b, half * T:(half + 1) * T, :] = res.results[core]["out"]
    return out
